# revision 40
# baseline (speedup 1.0000x reference)
"""Newton-Schulz matrix square root (nn_ASQRT) on 8 TRN2 NeuronCores.

Input  A: [32, 32, 128, 128] fp32 SPD matrices.
Output sA matching the 5-step coupled Newton-Schulz reference.

Data-parallel: 1024 matrices sharded 128 per core across 8 cores.

Per-matrix algebra (all iterates commute, symmetric):
    V0 = 0.5*A/nrm ; T0 = 1.5I - V0 ; Yh0 = V0
    u_n  = T_n^2 - 1.5 T_n          (note u0 = V0^2 - 1.5 V0)
    T_{n+1} = 1.5I + T_n u_n
    Yh_{n+1} = T_n Yh_n
    out = Yh_5 * 2*sqrt(nrm)

Sign trick at iter 0: u0 == -Yh1, so u0 is written straight into the
Yh slot of ty1 and the final scale is negated (no iter-0 Yh copy, no
T0 materialization; MM-A0/MM-B0 use V0 as stationary).

Matmul economics on TRN2: measured MM cadence is ~(6+FD)/2.4GHz + ~5ns
with the per-MM LDWEIGHTS (~92ns) overlapping the previous MM, so lone
128-wide MMs run LDW-bound at ~61-67ns and wider MMs at stream rate.
All matmul tiles are f16/bf16 (1 cycle/row). PSUM accumulation is f32;
elementwise STT/copies read f32 PSUM and write f16 SBUF.

Seeding scheme: the itB +-1.5I seed is a 256-wide const matmul per
j-pair writing ONLY the strided T'-halves of each psb bank
(start=True clears the bank's has_written, so the Y'-halves are
plain-written by the per-j accumulating matmul -- no idz zero-fill).
it0's T1 and the final scale are DVE STTs, not PE folds:
  T1  = 1.5*u0 - (cm15b-seed + V0.u0)      (MULTIPLY,SUBTRACT)
  out = psf * s2_broadcast (3 j) + ACT AP-scale copy (1 j)

GpSimd cannot touch PSUM on TRN2, so PSUM evacuation is DVE+ACT only:
iter T'|Y' copies run on ACT (psb fully folded); u-STTs, T1, finB and
the rowsq free-dim reduce run on DVE; A*A squares and V0 run on GpSimd
as single 512-el ops. finA+finB share tick 11 (psf is ready mid-tick
on PE; the fin STT sits at the DVE queue tail).

Measured engine busy per 4-matrix group/tick (ns), HW exec 166.5us:
  PE ~4160 | DVE ~4190 | ACT ~4000 | GpS ~2060
(DVE is gapless in steady state; remaining time is ~10us fixed NEFF
start/stop and ~20us pipeline fill/drain. Two-groups-per-tick fill
compression (ASQRT_FASTF) and a depth-10 schedule were measured
slower; partition_all_reduce for psn was 3x slower overall.)
"""
import os
import sys

sys.path.insert(0, "/opt/trn_rl_repo")

from contextlib import ExitStack

import numpy as np

B_S, C_DIM, N = 32, 32, 128
NCORES = 8
NMAT = int(os.environ.get("ASQRT_NMAT", str((B_S * C_DIM) // NCORES)))
GRP = 4                         # matrices per fused op / PSUM tile
NUM_ITER = 5

_CACHE = {}
LAST_EXEC_NS = None


def const_inputs():
    import ml_dtypes

    ident = np.eye(N, dtype=np.float32)
    return {
        "c15b": (1.5 * ident).astype(ml_dtypes.bfloat16),
        "cm15b": (-1.5 * ident).astype(ml_dtypes.bfloat16),
        "identb": ident.astype(ml_dtypes.bfloat16),
    }


def _build(dt_mm_name: str):
    import concourse.bacc as bacc
    import concourse.tile as tile
    import concourse.mybir as mybir

    F32 = mybir.dt.float32
    F16 = mybir.dt.float16
    AF = mybir.ActivationFunctionType
    ALU = mybir.AluOpType
    AX = mybir.AxisListType

    nc = bacc.Bacc(trn_type="TRN2", target_bir_lowering=False, debug=False)
    a = nc.dram_tensor("a", [NMAT, N, N], F32, kind="ExternalInput").ap()
    c15b = nc.dram_tensor("c15b", [N, N], mybir.dt.bfloat16, kind="ExternalInput").ap()
    cm15b = nc.dram_tensor("cm15b", [N, N], mybir.dt.bfloat16, kind="ExternalInput").ap()
    identb = nc.dram_tensor("identb", [N, N], mybir.dt.bfloat16, kind="ExternalInput").ap()
    o = nc.dram_tensor("o", [NMAT, N, N], F32, kind="ExternalOutput").ap()

    with tile.TileContext(nc) as tc, ExitStack() as ctx:
        cpool = ctx.enter_context(tc.tile_pool(name="consts", bufs=1))
        a_pool = ctx.enter_context(tc.tile_pool(name="a", bufs=10))
        v0_pool = ctx.enter_context(tc.tile_pool(name="v0", bufs=4))
        ty_pool = ctx.enter_context(tc.tile_pool(name="ty", bufs=18))
        sq_pool = ctx.enter_context(tc.tile_pool(name="sq", bufs=4))
        out_pool = ctx.enter_context(tc.tile_pool(name="out", bufs=4))
        nrm_pool = ctx.enter_context(tc.tile_pool(name="nrm", bufs=6))
        s2_pool = ctx.enter_context(tc.tile_pool(name="s2p", bufs=16))
        psa_pool = ctx.enter_context(tc.tile_pool(name="psa", bufs=4, space="PSUM"))
        psb_pool = ctx.enter_context(tc.tile_pool(name="psb", bufs=2, space="PSUM"))

        c15bt = cpool.tile([N, N], mybir.dt.bfloat16, tag="c15b")
        nc.sync.dma_start(c15bt[:], c15b)
        cm15bt = cpool.tile([N, N], mybir.dt.bfloat16, tag="cm15b")
        nc.sync.dma_start(cm15bt[:], cm15b)
        idb = cpool.tile([N, N], mybir.dt.bfloat16, tag="idb")
        nc.sync.dma_start(idb[:], identb)
        onest = cpool.tile([N, N], F32, tag="onest")
        nc.vector.memset(onest[:], 1.0)

        ngrp = NMAT // GRP

        st = {}  # per-group state

        def emit_dma(g):
            base = g * GRP
            ag = a_pool.tile([N, GRP, N], F32, tag="aq", name=f"aq{base}")
            nc.sync.dma_start(
                ag[:], a[base : base + GRP].rearrange("b p f -> p b f")
            )
            st[g] = {"ag": ag}

        RAMPG = int(os.environ.get("ASQRT_RAMPG", "0"))

        def emit_rowsq(g):
            base = g * GRP
            s = st[g]
            sq = sq_pool.tile([N, GRP, N], F16, tag="sq", name=f"sq{base}")
            rsg = nrm_pool.tile([N, GRP], F32, tag="rs", name=f"rs{base}")
            if g < RAMPG:
                # pipeline ramp: ACT is idle early, DVE is the steady-state
                # bottleneck -- run rowsq as per-j Square+accum on ACT
                for j in range(GRP):
                    nc.scalar.activation(
                        sq[:, j, :], s["ag"][:, j, :], AF.Square,
                        accum_out=rsg[:, j : j + 1],
                    )
            else:
                # steady state: f16 squares + pairwise fold on GpS halve the
                # DVE reduce width (DVE is the steady-state bottleneck)
                nc.gpsimd.tensor_tensor(sq[:], s["ag"][:], s["ag"][:], ALU.mult)
                sq2 = sq_pool.tile(
                    [N, GRP, N // 2], F32, tag="sq2", name=f"sq2{base}"
                )
                nc.gpsimd.tensor_tensor(
                    sq2[:], sq[:, :, 0 : N : 2], sq[:, :, 1 : N : 2], ALU.add
                )
                nc.vector.tensor_reduce(rsg[:], sq2[:], axis=AX.X, op=ALU.add)
            s["rsg"] = rsg

        def emit_norm(g):
            """PE norm broadcast + ACT/DVE scalar chain (V0 emitted later)."""
            base = g * GRP
            psn = psa_pool.tile([N, GRP], F32, tag="psa", name=f"psn{base}")
            nc.tensor.matmul(
                psn[:], lhsT=onest[:], rhs=st[g].pop("rsg"), start=True, stop=True
            )
            s = st[g]
            nrm2 = nrm_pool.tile([N, GRP], F32, tag="nrm2", name=f"nrm2{base}")
            nc.scalar.activation(nrm2[:], psn[:], AF.Sqrt, scale=4.0)  # 2*||A||
            s2 = s2_pool.tile([N, GRP], F32, tag="s2", name=f"s2{base}")
            nc.scalar.activation(s2[:], nrm2[:], AF.Sqrt, scale=2.0)   # 2*sqrt||A||
            rcp = nrm_pool.tile([N, GRP], F32, tag="rcp", name=f"rcp{base}")
            nc.vector.reciprocal(rcp[:], nrm2[:])                      # 0.5/||A||
            s["rcp"], s["s2"] = rcp, s2

        def emit_v0(g):
            base = g * GRP
            s = st[g]
            rcp = s.pop("rcp")
            v0 = v0_pool.tile([N, GRP, N], F16, tag="v0", name=f"v0{base}")
            nc.gpsimd.tensor_tensor(
                v0[:], s["ag"][:],
                rcp.unsqueeze(2).broadcast_to([N, GRP, N]),
                ALU.mult,
            )
            s.pop("ag")
            s["v0"] = v0

        def emit_it0A(g):
            base = g * GRP
            s = st[g]
            v0 = s["v0"]
            psa = psa_pool.tile([N, GRP, N], F32, tag="psa", name=f"psa{base}_0")
            for j in range(GRP):
                nc.tensor.matmul(
                    psa[:, j, :], lhsT=v0[:, j, :], rhs=v0[:, j, :],
                    start=True, stop=True,
                )
            # ty layout: [T | u | Ytilde] (3N wide)
            ty = ty_pool.tile([N, GRP, 3 * N], F16, tag="ty", name=f"ty{base}_1")
            # u0 = V0^2 - 1.5 V0 -> ty1 Y-section (== -Yh1, sign folded at out)
            nc.vector.scalar_tensor_tensor(
                out=ty[:, :, 2 * N :], in0=v0[:], scalar=-1.5,
                in1=psa[:], op0=ALU.mult, op1=ALU.add,
            )
            s["ty"] = ty

        def emit_it0B(g):
            base = g * GRP
            s = st[g]
            v0, ty = s.pop("v0"), s["ty"]
            psb = psa_pool.tile([N, GRP, N], F32, tag="psa", name=f"psb{base}_0")
            nc.tensor.matmul(  # psb = -1.5I on all j (512-wide const seed)
                psb[:], lhsT=cm15bt[:],
                rhs=idb[:].unsqueeze(1).broadcast_to([N, GRP, N]),
                start=True, stop=False, skip_group_check=True,
            )
            for j in range(GRP):
                nc.tensor.matmul(  # += V0 u0  => psb = -1.5I + V0u0
                    psb[:, j, :], lhsT=v0[:, j, :], rhs=ty[:, j, 2 * N :],
                    start=False, stop=True, skip_group_check=True,
                )
            # T1 = 1.5I + 1.5u0 - V0u0 = 1.5*u0 - psb
            nc.vector.scalar_tensor_tensor(
                out=ty[:, :, 0:N], in0=ty[:, :, 2 * N :], scalar=1.5,
                in1=psb[:], op0=ALU.mult, op1=ALU.subtract,
            )

        def emit_itA(g, it):
            base = g * GRP
            s = st[g]
            ty = s["ty"]
            psa = psa_pool.tile(
                [N, GRP, N], F32, tag="psa", name=f"psa{base}_{it}"
            )
            for j in range(GRP):
                nc.tensor.matmul(
                    psa[:, j, :], lhsT=ty[:, j, 0:N], rhs=ty[:, j, 0:N],
                    start=True, stop=True,
                )
            if it < NUM_ITER - 2:
                nc.vector.scalar_tensor_tensor(
                    out=ty[:, :, N : 2 * N], in0=ty[:, :, 0:N], scalar=-1.5,
                    in1=psa[:], op0=ALU.mult, op1=ALU.add,
                )
            else:
                # u3 = 1.5 T - T^2 = -u: makes psb3 = -1.5I - T u = -T4 so
                # psf = (-T4)(-Yh4) = +Yh5 and the out scale is +2 sqrt(nrm)
                nc.vector.scalar_tensor_tensor(
                    out=ty[:, :, N : 2 * N], in0=ty[:, :, 0:N], scalar=1.5,
                    in1=psa[:], op0=ALU.mult, op1=ALU.subtract,
                )
            tyn = ty_pool.tile(
                [N, GRP, 3 * N], F16, tag="ty", name=f"ty{base}_{it + 1}"
            )
            s["tyn"] = tyn

        def emit_itB(g, it):
            base = g * GRP
            s = st[g]
            ty, tyn = s["ty"], s["tyn"]
            psb = psb_pool.tile(
                [N, GRP, 2 * N], F32, tag="psb", name=f"psb{base}_{it}"
            )
            seed = c15bt if it < NUM_ITER - 2 else cm15bt
            for h in range(2):  # seed halves: matmul out must stay in-bank
                nc.tensor.matmul(  # psb T'-halves = +-1.5I on j pair
                    psb[:, 2 * h : 2 * h + 2, 0:N], lhsT=seed[:],
                    rhs=idb[:].unsqueeze(1).broadcast_to([N, 2, N]),
                    start=True, stop=False, skip_group_check=True,
                )
            for j in range(GRP):
                nc.tensor.matmul(  # += T.[u|Y] => psb = [T' | Y']
                    psb[:, j, :], lhsT=ty[:, j, 0:N], rhs=ty[:, j, N:],
                    start=False, stop=True, skip_group_check=True,
                )
            # one copy: T' -> tyn[0:N], Y' -> tyn[2N:3N] (2-chunk out AP)
            tyn_tu = tyn[:, :, 0 : 3 * N].rearrange(
                "p b (c n) -> p b c n", c=3
            )[:, :, 0::2, :]
            nc.scalar.copy(tyn_tu, psb[:])
            s["ty"] = tyn
            del s["tyn"]

        def emit_finA(g):
            base = g * GRP
            s = st[g]
            ty = s.pop("ty")
            psf = psa_pool.tile([N, GRP, N], F32, tag="psa", name=f"psf{base}")
            for j in range(GRP):
                nc.tensor.matmul(
                    psf[:, j, :], lhsT=ty[:, j, 0:N], rhs=ty[:, j, 2 * N :],
                    start=True, stop=True,
                )
            s["psf"] = psf

        def emit_finB(g):
            base = g * GRP
            s = st.pop(g)
            psf, s2 = s["psf"], s["s2"]
            outg = out_pool.tile([N, GRP, N], F32, tag="outq", name=f"out{base}")
            # out = +2*sqrt(nrm) * psf  (signs cancel: psf = (-T4)(-Yh4))
            # split 3 j on DVE / 1 j on ACT to balance the two engines
            nc.vector.scalar_tensor_tensor(
                out=outg[:, 0:3, :], in0=psf[:, 0:3, :], scalar=1.0,
                in1=s2[:, 0:3].unsqueeze(2).broadcast_to([N, 3, N]),
                op0=ALU.mult, op1=ALU.mult,
            )
            nc.scalar.activation(
                outg[:, 3, :], psf[:, 3, :], AF.Copy, scale=s2[:, 3:4],
            )
            nc.sync.dma_start(
                o[base : base + GRP].rearrange("b p f -> p b f"), outg[:]
            )

        # --- staggered pipeline ------------------------------------------
        # offsets: dma@0 rowsq@1 norm@2 it0A@3 it0B@4 it1A@5 it1B@6
        #          it2A@7 it2B@8 it3A@9 it3B@10 finA+finB@11
        # finA's psf is consumed by finB's STT at the DVE queue tail, so
        # both fit in one tick (psf is ready ~mid-tick on PE, the STT runs
        # last on DVE) -- one less pipeline stage of fill/drain.
        # Emission order within a tick shapes each engine's queue: finB
        # first (frees psf), then rowsq/norm scalars, the A-stages (psa
        # producers early), B-stages, finA, V0 late (GpS tail), dma last.
        DEPTH = 11
        # Fill compression: the first FASTF groups enter two per tick, so
        # the bottleneck engines saturate early in the ramp instead of
        # idling while group 0 trickles through the 12-stage chain. The
        # execution is self-timed dataflow; overfilled early ticks just
        # backpressure through the tile pools.
        FASTF = int(os.environ.get("ASQRT_FASTF", "0"))
        FASTF = min(FASTF - FASTF % 2, ngrp)

        def start_tick(g):
            return g // 2 if g < FASTF else g - FASTF // 2

        from collections import defaultdict

        tick_groups = defaultdict(list)
        for g in range(ngrp):
            tick_groups[start_tick(g)].append(g)
        last = start_tick(ngrp - 1)

        stages = [
            (emit_it0A, 3),
            (emit_rowsq, 1),
            (emit_norm, 2),
            (lambda g: emit_itA(g, 1), 5),
            (lambda g: emit_itA(g, 2), 7),
            (lambda g: emit_itA(g, 3), 9),
            (emit_it0B, 4),
            (lambda g: emit_itB(g, 1), 6),
            (lambda g: emit_itB(g, 2), 8),
            (lambda g: emit_itB(g, 3), 10),
            (emit_finA, 11),
            (emit_v0, 2),
            (emit_finB, 11),
            (emit_dma, 0),
        ]
        for t in range(last + DEPTH + 1):
            for fn, off in stages:
                for g in tick_groups.get(t - off, ()):
                    fn(g)

    nc.compile()
    return nc


def _get_nc():
    dt_mm = os.environ.get("ASQRT_DTYPE", "f32r")
    if dt_mm not in _CACHE:
        _CACHE[dt_mm] = _build(dt_mm)
    return _CACHE[dt_mm]


def kernel(A: np.ndarray) -> np.ndarray:
    global LAST_EXEC_NS
    from concourse.bass_utils import run_bass_kernel_spmd

    nc = _get_nc()
    A2 = np.ascontiguousarray(A.reshape(-1, N, N), dtype=np.float32)
    consts = const_inputs()
    in_maps = [
        {"a": A2[i * NMAT : (i + 1) * NMAT], **consts}
        for i in range(NCORES)
    ]
    trace = os.environ.get("ASQRT_TRACE", "0") == "1"
    res = run_bass_kernel_spmd(nc, in_maps, list(range(NCORES)), trace=trace)
    LAST_EXEC_NS = res.exec_time_ns
    out = np.concatenate([r["o"] for r in res.results], axis=0)
    return out.reshape(B_S, C_DIM, N, N)


if __name__ == "__main__":
    rng = np.random.default_rng(0)
    A = rng.standard_normal((B_S, C_DIM, N, N)).astype(np.float32)
    A = np.einsum("bcij,bckj->bcik", A, A) / N + 1e-3 * np.eye(N, dtype=np.float32)
    out = kernel(A)
    print("ok", out.shape, LAST_EXEC_NS)


# revision 41
# speedup vs baseline: 1.0512x; 1.0512x over previous
"""Newton-Schulz matrix square root (nn_ASQRT) on 8 TRN2 NeuronCores.

Input  A: [32, 32, 128, 128] fp32 SPD matrices.
Output sA matching the 5-step coupled Newton-Schulz reference.

Data-parallel: 1024 matrices sharded 128 per core across 8 cores.

Per-matrix algebra (all iterates commute, symmetric):
    V0 = 0.5*A/nrm ; T0 = 1.5I - V0 ; Yh0 = V0
    u_n  = T_n^2 - 1.5 T_n          (note u0 = V0^2 - 1.5 V0)
    T_{n+1} = 1.5I + T_n u_n
    Yh_{n+1} = T_n Yh_n
    out = Yh_5 * 2*sqrt(nrm)

Sign trick at iter 0: u0 == -Yh1, so u0 is written straight into the
Yh slot of ty1 and the final scale is negated (no iter-0 Yh copy, no
T0 materialization; MM-A0/MM-B0 use V0 as stationary).

Matmul economics on TRN2: measured MM cadence is ~(6+FD)/2.4GHz + ~5ns
with the per-MM LDWEIGHTS (~92ns) overlapping the previous MM, so lone
128-wide MMs run LDW-bound at ~61-67ns and wider MMs at stream rate.
All matmul tiles are f16/bf16 (1 cycle/row). PSUM accumulation is f32;
elementwise STT/copies read f32 PSUM and write f16 SBUF.

Seeding scheme: the itB +-1.5I seed is a 256-wide const matmul per
j-pair writing ONLY the strided T'-halves of each psb bank
(start=True clears the bank's has_written, so the Y'-halves are
plain-written by the per-j accumulating matmul -- no idz zero-fill).
it0's T1 and the final scale are DVE STTs, not PE folds:
  T1  = 1.5*u0 - (cm15b-seed + V0.u0)      (MULTIPLY,SUBTRACT)
  out = psf * s2_broadcast (3 j) + ACT AP-scale copy (1 j)

GpSimd cannot touch PSUM on TRN2, so PSUM evacuation is DVE+ACT only:
iter T'|Y' copies run on ACT (psb fully folded); u-STTs, T1, finB and
the rowsq free-dim reduce run on DVE; A*A squares and V0 run on GpSimd
as single 512-el ops. finA+finB share tick 11 (psf is ready mid-tick
on PE; the fin STT sits at the DVE queue tail).

Measured engine busy per 4-matrix group/tick (ns), HW exec 166.5us:
  PE ~4160 | DVE ~4190 | ACT ~4000 | GpS ~2060
(DVE is gapless in steady state; remaining time is ~10us fixed NEFF
start/stop and ~20us pipeline fill/drain. Two-groups-per-tick fill
compression (ASQRT_FASTF) and a depth-10 schedule were measured
slower; partition_all_reduce for psn was 3x slower overall.)
"""
import os
import sys

sys.path.insert(0, "/opt/trn_rl_repo")

from contextlib import ExitStack

import numpy as np

B_S, C_DIM, N = 32, 32, 128
NCORES = 8
NMAT = int(os.environ.get("ASQRT_NMAT", str((B_S * C_DIM) // NCORES)))
GRP = 4                         # matrices per fused op / PSUM tile
NUM_ITER = 5

_CACHE = {}
LAST_EXEC_NS = None


def const_inputs():
    import ml_dtypes

    ident = np.eye(N, dtype=np.float32)
    return {
        "c15b": (1.5 * ident).astype(ml_dtypes.bfloat16),
        "cm15b": (-1.5 * ident).astype(ml_dtypes.bfloat16),
        "identb": ident.astype(ml_dtypes.bfloat16),
    }


def _build(dt_mm_name: str):
    import concourse.bacc as bacc
    import concourse.tile as tile
    import concourse.mybir as mybir

    F32 = mybir.dt.float32
    F16 = mybir.dt.float16
    AF = mybir.ActivationFunctionType
    ALU = mybir.AluOpType
    AX = mybir.AxisListType

    nc = bacc.Bacc(trn_type="TRN2", target_bir_lowering=False, debug=False)
    a = nc.dram_tensor("a", [NMAT, N, N], F32, kind="ExternalInput").ap()
    c15b = nc.dram_tensor("c15b", [N, N], mybir.dt.bfloat16, kind="ExternalInput").ap()
    cm15b = nc.dram_tensor("cm15b", [N, N], mybir.dt.bfloat16, kind="ExternalInput").ap()
    identb = nc.dram_tensor("identb", [N, N], mybir.dt.bfloat16, kind="ExternalInput").ap()
    o = nc.dram_tensor("o", [NMAT, N, N], F32, kind="ExternalOutput").ap()

    with tile.TileContext(nc) as tc, ExitStack() as ctx:
        cpool = ctx.enter_context(tc.tile_pool(name="consts", bufs=1))
        a_pool = ctx.enter_context(tc.tile_pool(name="a", bufs=10))
        v0_pool = ctx.enter_context(tc.tile_pool(name="v0", bufs=4))
        ty_pool = ctx.enter_context(tc.tile_pool(name="ty", bufs=18))
        sq_pool = ctx.enter_context(tc.tile_pool(name="sq", bufs=4))
        out_pool = ctx.enter_context(tc.tile_pool(name="out", bufs=4))
        nrm_pool = ctx.enter_context(tc.tile_pool(name="nrm", bufs=6))
        s2_pool = ctx.enter_context(tc.tile_pool(name="s2p", bufs=16))
        psa_pool = ctx.enter_context(tc.tile_pool(name="psa", bufs=4, space="PSUM"))
        psb_pool = ctx.enter_context(tc.tile_pool(name="psb", bufs=2, space="PSUM"))

        c15bt = cpool.tile([N, N], mybir.dt.bfloat16, tag="c15b")
        nc.sync.dma_start(c15bt[:], c15b)
        cm15bt = cpool.tile([N, N], mybir.dt.bfloat16, tag="cm15b")
        nc.sync.dma_start(cm15bt[:], cm15b)
        idb = cpool.tile([N, N], mybir.dt.bfloat16, tag="idb")
        nc.sync.dma_start(idb[:], identb)
        onest = cpool.tile([N, N], F32, tag="onest")
        nc.vector.memset(onest[:], 1.0)

        ngrp = NMAT // GRP

        st = {}  # per-group state

        def emit_dma(g):
            base = g * GRP
            ag = a_pool.tile([N, GRP, N], F32, tag="aq", name=f"aq{base}")
            nc.sync.dma_start(
                ag[:], a[base : base + GRP].rearrange("b p f -> p b f")
            )
            st[g] = {"ag": ag}

        RAMPG = int(os.environ.get("ASQRT_RAMPG", "0"))

        def emit_rowsq(g):
            base = g * GRP
            s = st[g]
            sq = sq_pool.tile([N, GRP, N], F16, tag="sq", name=f"sq{base}")
            rsg = nrm_pool.tile([N, GRP], F32, tag="rs", name=f"rs{base}")
            if g < RAMPG:
                # pipeline ramp: ACT is idle early, DVE is the steady-state
                # bottleneck -- run rowsq as per-j Square+accum on ACT
                for j in range(GRP):
                    nc.scalar.activation(
                        sq[:, j, :], s["ag"][:, j, :], AF.Square,
                        accum_out=rsg[:, j : j + 1],
                    )
            else:
                # steady state: f16 squares + pairwise fold on GpS halve the
                # DVE reduce width (DVE is the steady-state bottleneck)
                nc.gpsimd.tensor_tensor(sq[:], s["ag"][:], s["ag"][:], ALU.mult)
                sq2 = sq_pool.tile(
                    [N, GRP, N // 2], F32, tag="sq2", name=f"sq2{base}"
                )
                nc.gpsimd.tensor_tensor(
                    sq2[:], sq[:, :, 0 : N : 2], sq[:, :, 1 : N : 2], ALU.add
                )
                nc.vector.tensor_reduce(rsg[:], sq2[:], axis=AX.X, op=ALU.add)
            s["rsg"] = rsg

        def emit_norm(g):
            """PE norm broadcast + ACT/DVE scalar chain (V0 emitted later)."""
            base = g * GRP
            psn = psa_pool.tile([N, GRP], F32, tag="psa", name=f"psn{base}")
            nc.tensor.matmul(
                psn[:], lhsT=onest[:], rhs=st[g].pop("rsg"), start=True, stop=True
            )
            s = st[g]
            nrm2 = nrm_pool.tile([N, GRP], F32, tag="nrm2", name=f"nrm2{base}")
            nc.scalar.activation(nrm2[:], psn[:], AF.Sqrt, scale=4.0)  # 2*||A||
            s2 = s2_pool.tile([N, GRP], F32, tag="s2", name=f"s2{base}")
            nc.scalar.activation(s2[:], nrm2[:], AF.Sqrt, scale=2.0)   # 2*sqrt||A||
            rcp = nrm_pool.tile([N, GRP], F32, tag="rcp", name=f"rcp{base}")
            nc.vector.reciprocal(rcp[:], nrm2[:])                      # 0.5/||A||
            s["rcp"], s["s2"] = rcp, s2

        def emit_v0(g):
            base = g * GRP
            s = st[g]
            rcp = s.pop("rcp")
            v0 = v0_pool.tile([N, GRP, N], F16, tag="v0", name=f"v0{base}")
            nc.gpsimd.tensor_tensor(
                v0[:], s["ag"][:],
                rcp.unsqueeze(2).broadcast_to([N, GRP, N]),
                ALU.mult,
            )
            s.pop("ag")
            s["v0"] = v0

        def emit_it0A(g):
            base = g * GRP
            s = st[g]
            v0 = s["v0"]
            psa = psa_pool.tile([N, GRP, N], F32, tag="psa", name=f"psa{base}_0")
            for j in range(GRP):
                nc.tensor.matmul(
                    psa[:, j, :], lhsT=v0[:, j, :], rhs=v0[:, j, :],
                    start=True, stop=True,
                )
            # ty layout: [T | u | Ytilde] (3N wide)
            ty = ty_pool.tile([N, GRP, 3 * N], F16, tag="ty", name=f"ty{base}_1")
            # u0 = V0^2 - 1.5 V0 -> ty1 Y-section (== -Yh1, sign folded at out)
            nc.vector.scalar_tensor_tensor(
                out=ty[:, :, 2 * N :], in0=v0[:], scalar=-1.5,
                in1=psa[:], op0=ALU.mult, op1=ALU.add,
            )
            s["ty"] = ty

        def emit_it0B(g):
            base = g * GRP
            s = st[g]
            v0, ty = s.pop("v0"), s["ty"]
            psb = psa_pool.tile([N, GRP, N], F32, tag="psa", name=f"psb{base}_0")
            nc.tensor.matmul(  # psb = -1.5I on all j (512-wide const seed)
                psb[:], lhsT=cm15bt[:],
                rhs=idb[:].unsqueeze(1).broadcast_to([N, GRP, N]),
                start=True, stop=False, skip_group_check=True,
            )
            for j in range(GRP):
                nc.tensor.matmul(  # += V0 u0  => psb = -1.5I + V0u0
                    psb[:, j, :], lhsT=v0[:, j, :], rhs=ty[:, j, 2 * N :],
                    start=False, stop=True, skip_group_check=True,
                )
            # T1 = 1.5I + 1.5u0 - V0u0 = 1.5*u0 - psb
            nc.vector.scalar_tensor_tensor(
                out=ty[:, :, 0:N], in0=ty[:, :, 2 * N :], scalar=1.5,
                in1=psb[:], op0=ALU.mult, op1=ALU.subtract,
            )

        def emit_itA(g, it):
            base = g * GRP
            s = st[g]
            ty = s["ty"]
            psa = psa_pool.tile(
                [N, GRP, N], F32, tag="psa", name=f"psa{base}_{it}"
            )
            for j in range(GRP):
                nc.tensor.matmul(
                    psa[:, j, :], lhsT=ty[:, j, 0:N], rhs=ty[:, j, 0:N],
                    start=True, stop=True,
                )
            if it < NUM_ITER - 2:
                nc.vector.scalar_tensor_tensor(
                    out=ty[:, :, N : 2 * N], in0=ty[:, :, 0:N], scalar=-1.5,
                    in1=psa[:], op0=ALU.mult, op1=ALU.add,
                )
            else:
                # u3 = 1.5 T - T^2 = -u: makes psb3 = -1.5I - T u = -T4 so
                # psf = (-T4)(-Yh4) = +Yh5 and the out scale is +2 sqrt(nrm)
                nc.vector.scalar_tensor_tensor(
                    out=ty[:, :, N : 2 * N], in0=ty[:, :, 0:N], scalar=1.5,
                    in1=psa[:], op0=ALU.mult, op1=ALU.subtract,
                )
            tyn = ty_pool.tile(
                [N, GRP, 3 * N], F16, tag="ty", name=f"ty{base}_{it + 1}"
            )
            s["tyn"] = tyn

        def emit_itB(g, it):
            base = g * GRP
            s = st[g]
            ty, tyn = s["ty"], s["tyn"]
            psb = psb_pool.tile(
                [N, GRP, 2 * N], F32, tag="psb", name=f"psb{base}_{it}"
            )
            seed = c15bt if it < NUM_ITER - 2 else cm15bt
            for h in range(2):  # seed halves: matmul out must stay in-bank
                nc.tensor.matmul(  # psb T'-halves = +-1.5I on j pair
                    psb[:, 2 * h : 2 * h + 2, 0:N], lhsT=seed[:],
                    rhs=idb[:].unsqueeze(1).broadcast_to([N, 2, N]),
                    start=True, stop=False, skip_group_check=True,
                )
            for j in range(GRP):
                nc.tensor.matmul(  # += T.[u|Y] => psb = [T' | Y']
                    psb[:, j, :], lhsT=ty[:, j, 0:N], rhs=ty[:, j, N:],
                    start=False, stop=True, skip_group_check=True,
                )
            # one copy: T' -> tyn[0:N], Y' -> tyn[2N:3N] (2-chunk out AP)
            tyn_tu = tyn[:, :, 0 : 3 * N].rearrange(
                "p b (c n) -> p b c n", c=3
            )[:, :, 0::2, :]
            nc.scalar.copy(tyn_tu, psb[:])
            s["ty"] = tyn
            del s["tyn"]

        def emit_finA(g):
            base = g * GRP
            s = st[g]
            ty = s.pop("ty")
            psf = psa_pool.tile([N, GRP, N], F32, tag="psa", name=f"psf{base}")
            for j in range(GRP):
                nc.tensor.matmul(
                    psf[:, j, :], lhsT=ty[:, j, 0:N], rhs=ty[:, j, 2 * N :],
                    start=True, stop=True,
                )
            s["psf"] = psf

        def emit_finB(g):
            base = g * GRP
            s = st.pop(g)
            psf, s2 = s["psf"], s["s2"]
            outg = out_pool.tile([N, GRP, N], F32, tag="outq", name=f"out{base}")
            # out = +2*sqrt(nrm) * psf  (signs cancel: psf = (-T4)(-Yh4))
            # split 3 j on DVE / 1 j on ACT to balance the two engines
            nc.vector.scalar_tensor_tensor(
                out=outg[:, 0:3, :], in0=psf[:, 0:3, :], scalar=1.0,
                in1=s2[:, 0:3].unsqueeze(2).broadcast_to([N, 3, N]),
                op0=ALU.mult, op1=ALU.mult,
            )
            nc.scalar.activation(
                outg[:, 3, :], psf[:, 3, :], AF.Copy, scale=s2[:, 3:4],
            )
            nc.sync.dma_start(
                o[base : base + GRP].rearrange("b p f -> p b f"), outg[:]
            )

        # --- staggered pipeline ------------------------------------------
        # offsets: dma@0 rowsq@1 norm@2 it0A@3 it0B@4 it1A@5 it1B@6
        #          it2A@7 it2B@8 it3A@9 it3B@10 finA+finB@11
        # finA's psf is consumed by finB's STT at the DVE queue tail, so
        # both fit in one tick (psf is ready ~mid-tick on PE, the STT runs
        # last on DVE) -- one less pipeline stage of fill/drain.
        # Emission order within a tick shapes each engine's queue: finB
        # first (frees psf), then rowsq/norm scalars, the A-stages (psa
        # producers early), B-stages, finA, V0 late (GpS tail), dma last.
        DEPTH = 11
        # Fill compression: the first FASTF groups enter two per tick, so
        # the bottleneck engines saturate early in the ramp instead of
        # idling while group 0 trickles through the 12-stage chain. The
        # execution is self-timed dataflow; overfilled early ticks just
        # backpressure through the tile pools.
        FASTF = int(os.environ.get("ASQRT_FASTF", "0"))
        FASTF = min(FASTF - FASTF % 2, ngrp)

        def start_tick(g):
            return g // 2 if g < FASTF else g - FASTF // 2

        from collections import defaultdict

        tick_groups = defaultdict(list)
        for g in range(ngrp):
            tick_groups[start_tick(g)].append(g)
        last = start_tick(ngrp - 1)

        stages = [
            (emit_rowsq, 1),
            (emit_norm, 2),
            (emit_it0A, 3),
            (lambda g: emit_itA(g, 1), 5),
            (lambda g: emit_itA(g, 2), 7),
            (lambda g: emit_itA(g, 3), 9),
            (emit_it0B, 4),
            (lambda g: emit_itB(g, 1), 6),
            (lambda g: emit_itB(g, 2), 8),
            (lambda g: emit_itB(g, 3), 10),
            (emit_finA, 11),
            (emit_v0, 2),
            (emit_finB, 11),
            (emit_dma, 0),
        ]
        for t in range(last + DEPTH + 1):
            for fn, off in stages:
                for g in tick_groups.get(t - off, ()):
                    fn(g)

    nc.compile()
    return nc


def _get_nc():
    dt_mm = os.environ.get("ASQRT_DTYPE", "f32r")
    if dt_mm not in _CACHE:
        _CACHE[dt_mm] = _build(dt_mm)
    return _CACHE[dt_mm]


def kernel(A: np.ndarray) -> np.ndarray:
    global LAST_EXEC_NS
    from concourse.bass_utils import run_bass_kernel_spmd

    nc = _get_nc()
    A2 = np.ascontiguousarray(A.reshape(-1, N, N), dtype=np.float32)
    consts = const_inputs()
    in_maps = [
        {"a": A2[i * NMAT : (i + 1) * NMAT], **consts}
        for i in range(NCORES)
    ]
    trace = os.environ.get("ASQRT_TRACE", "0") == "1"
    res = run_bass_kernel_spmd(nc, in_maps, list(range(NCORES)), trace=trace)
    LAST_EXEC_NS = res.exec_time_ns
    out = np.concatenate([r["o"] for r in res.results], axis=0)
    return out.reshape(B_S, C_DIM, N, N)


if __name__ == "__main__":
    rng = np.random.default_rng(0)
    A = rng.standard_normal((B_S, C_DIM, N, N)).astype(np.float32)
    A = np.einsum("bcij,bckj->bcik", A, A) / N + 1e-3 * np.eye(N, dtype=np.float32)
    out = kernel(A)
    print("ok", out.shape, LAST_EXEC_NS)


# revision 42
# speedup vs baseline: 1.0630x; 1.0112x over previous
"""Newton-Schulz matrix square root (nn_ASQRT) on 8 TRN2 NeuronCores.

Input  A: [32, 32, 128, 128] fp32 SPD matrices.
Output sA matching the 5-step coupled Newton-Schulz reference.

Data-parallel: 1024 matrices sharded 128 per core across 8 cores.

Per-matrix algebra (all iterates commute, symmetric):
    V0 = 0.5*A/nrm ; T0 = 1.5I - V0 ; Yh0 = V0
    u_n  = T_n^2 - 1.5 T_n          (note u0 = V0^2 - 1.5 V0)
    T_{n+1} = 1.5I + T_n u_n
    Yh_{n+1} = T_n Yh_n
    out = Yh_5 * 2*sqrt(nrm)

Sign trick at iter 0: u0 == -Yh1, so u0 is written straight into the
Yh slot of ty1 and the final scale is negated (no iter-0 Yh copy, no
T0 materialization; MM-A0/MM-B0 use V0 as stationary).

Matmul economics on TRN2: measured MM cadence is ~(6+FD)/2.4GHz + ~5ns
with the per-MM LDWEIGHTS (~92ns) overlapping the previous MM, so lone
128-wide MMs run LDW-bound at ~61-67ns and wider MMs at stream rate.
All matmul tiles are f16/bf16 (1 cycle/row). PSUM accumulation is f32;
elementwise STT/copies read f32 PSUM and write f16 SBUF.

Seeding scheme: the itB +-1.5I seed is a 256-wide const matmul per
j-pair writing ONLY the strided T'-halves of each psb bank
(start=True clears the bank's has_written, so the Y'-halves are
plain-written by the per-j accumulating matmul -- no idz zero-fill).
it0's T1 and the final scale are DVE STTs, not PE folds:
  T1  = 1.5*u0 - (cm15b-seed + V0.u0)      (MULTIPLY,SUBTRACT)
  out = psf * s2_broadcast (3 j) + ACT AP-scale copy (1 j)

GpSimd cannot touch PSUM on TRN2, so PSUM evacuation is DVE+ACT only:
iter T'|Y' copies run on ACT (psb fully folded); u-STTs, T1, finB and
the rowsq free-dim reduce run on DVE; A*A squares and V0 run on GpSimd
as single 512-el ops. finA+finB share tick 11 (psf is ready mid-tick
on PE; the fin STT sits at the DVE queue tail).

Measured engine busy per 4-matrix group/tick (ns), HW exec 166.5us:
  PE ~4160 | DVE ~4190 | ACT ~4000 | GpS ~2060
(DVE is gapless in steady state; remaining time is ~10us fixed NEFF
start/stop and ~20us pipeline fill/drain. Two-groups-per-tick fill
compression (ASQRT_FASTF) and a depth-10 schedule were measured
slower; partition_all_reduce for psn was 3x slower overall.)
"""
import os
import sys

sys.path.insert(0, "/opt/trn_rl_repo")

from contextlib import ExitStack

import numpy as np

B_S, C_DIM, N = 32, 32, 128
NCORES = 8
NMAT = int(os.environ.get("ASQRT_NMAT", str((B_S * C_DIM) // NCORES)))
GRP = 4                         # matrices per fused op / PSUM tile
NUM_ITER = 5

_CACHE = {}
LAST_EXEC_NS = None


def const_inputs():
    import ml_dtypes

    ident = np.eye(N, dtype=np.float32)
    return {
        "c15b": (1.5 * ident).astype(ml_dtypes.bfloat16),
        "cm15b": (-1.5 * ident).astype(ml_dtypes.bfloat16),
        "identb": ident.astype(ml_dtypes.bfloat16),
    }


def _build(dt_mm_name: str):
    import concourse.bacc as bacc
    import concourse.tile as tile
    import concourse.mybir as mybir

    F32 = mybir.dt.float32
    F16 = mybir.dt.float16
    AF = mybir.ActivationFunctionType
    ALU = mybir.AluOpType
    AX = mybir.AxisListType

    nc = bacc.Bacc(trn_type="TRN2", target_bir_lowering=False, debug=False)
    a = nc.dram_tensor("a", [NMAT, N, N], F32, kind="ExternalInput").ap()
    c15b = nc.dram_tensor("c15b", [N, N], mybir.dt.bfloat16, kind="ExternalInput").ap()
    cm15b = nc.dram_tensor("cm15b", [N, N], mybir.dt.bfloat16, kind="ExternalInput").ap()
    identb = nc.dram_tensor("identb", [N, N], mybir.dt.bfloat16, kind="ExternalInput").ap()
    o = nc.dram_tensor("o", [NMAT, N, N], F32, kind="ExternalOutput").ap()

    with tile.TileContext(nc) as tc, ExitStack() as ctx:
        cpool = ctx.enter_context(tc.tile_pool(name="consts", bufs=1))
        a_pool = ctx.enter_context(tc.tile_pool(name="a", bufs=10))
        v0_pool = ctx.enter_context(tc.tile_pool(name="v0", bufs=4))
        ty_pool = ctx.enter_context(tc.tile_pool(name="ty", bufs=18))
        sq_pool = ctx.enter_context(tc.tile_pool(name="sq", bufs=4))
        out_pool = ctx.enter_context(tc.tile_pool(name="out", bufs=4))
        nrm_pool = ctx.enter_context(tc.tile_pool(name="nrm", bufs=6))
        s2_pool = ctx.enter_context(tc.tile_pool(name="s2p", bufs=16))
        psa_pool = ctx.enter_context(tc.tile_pool(name="psa", bufs=4, space="PSUM"))
        psb_pool = ctx.enter_context(tc.tile_pool(name="psb", bufs=2, space="PSUM"))

        c15bt = cpool.tile([N, N], mybir.dt.bfloat16, tag="c15b")
        nc.sync.dma_start(c15bt[:], c15b)
        cm15bt = cpool.tile([N, N], mybir.dt.bfloat16, tag="cm15b")
        nc.sync.dma_start(cm15bt[:], cm15b)
        idb = cpool.tile([N, N], mybir.dt.bfloat16, tag="idb")
        nc.sync.dma_start(idb[:], identb)
        onest = cpool.tile([N, N], F32, tag="onest")
        nc.vector.memset(onest[:], 1.0)

        ngrp = NMAT // GRP

        st = {}  # per-group state

        def emit_dma(g):
            base = g * GRP
            ag = a_pool.tile([N, GRP, N], F32, tag="aq", name=f"aq{base}")
            nc.sync.dma_start(
                ag[:], a[base : base + GRP].rearrange("b p f -> p b f")
            )
            st[g] = {"ag": ag}

        RAMPG = int(os.environ.get("ASQRT_RAMPG", "0"))

        def emit_rowsq(g):
            base = g * GRP
            s = st[g]
            sq = sq_pool.tile([N, GRP, N], F16, tag="sq", name=f"sq{base}")
            rsg = nrm_pool.tile([N, GRP], F32, tag="rs", name=f"rs{base}")
            if g < RAMPG:
                # pipeline ramp: ACT is idle early, DVE is the steady-state
                # bottleneck -- run rowsq as per-j Square+accum on ACT
                for j in range(GRP):
                    nc.scalar.activation(
                        sq[:, j, :], s["ag"][:, j, :], AF.Square,
                        accum_out=rsg[:, j : j + 1],
                    )
            else:
                # steady state: f16 squares on GpS; free-dim sum on DVE
                # (the only free-dim reducer that can keep up)
                nc.gpsimd.tensor_tensor(sq[:], s["ag"][:], s["ag"][:], ALU.mult)
                nc.vector.tensor_reduce(rsg[:], sq[:], axis=AX.X, op=ALU.add)
            s["rsg"] = rsg

        def emit_norm(g):
            """PE norm broadcast + ACT/DVE scalar chain (V0 emitted later)."""
            base = g * GRP
            psn = psa_pool.tile([N, GRP], F32, tag="psa", name=f"psn{base}")
            nc.tensor.matmul(
                psn[:], lhsT=onest[:], rhs=st[g].pop("rsg"), start=True, stop=True
            )
            s = st[g]
            nrm2 = nrm_pool.tile([N, GRP], F32, tag="nrm2", name=f"nrm2{base}")
            nc.scalar.activation(nrm2[:], psn[:], AF.Sqrt, scale=4.0)  # 2*||A||
            s2 = s2_pool.tile([N, GRP], F32, tag="s2", name=f"s2{base}")
            nc.scalar.activation(s2[:], nrm2[:], AF.Sqrt, scale=2.0)   # 2*sqrt||A||
            rcp = nrm_pool.tile([N, GRP], F32, tag="rcp", name=f"rcp{base}")
            nc.vector.reciprocal(rcp[:], nrm2[:])                      # 0.5/||A||
            s["rcp"], s["s2"] = rcp, s2

        def emit_v0(g):
            base = g * GRP
            s = st[g]
            rcp = s.pop("rcp")
            v0 = v0_pool.tile([N, GRP, N], F16, tag="v0", name=f"v0{base}")
            nc.gpsimd.tensor_tensor(
                v0[:], s["ag"][:],
                rcp.unsqueeze(2).broadcast_to([N, GRP, N]),
                ALU.mult,
            )
            s.pop("ag")
            s["v0"] = v0

        def emit_it0A(g):
            base = g * GRP
            s = st[g]
            v0 = s["v0"]
            psa = psa_pool.tile([N, GRP, N], F32, tag="psa", name=f"psa{base}_0")
            for j in range(GRP):
                nc.tensor.matmul(
                    psa[:, j, :], lhsT=v0[:, j, :], rhs=v0[:, j, :],
                    start=True, stop=True,
                )
            # ty layout: [T | u | Ytilde] (3N wide)
            ty = ty_pool.tile([N, GRP, 3 * N], F16, tag="ty", name=f"ty{base}_1")
            # u0 = V0^2 - 1.5 V0 -> ty1 Y-section (== -Yh1, sign folded at out)
            nc.vector.scalar_tensor_tensor(
                out=ty[:, :, 2 * N :], in0=v0[:], scalar=-1.5,
                in1=psa[:], op0=ALU.mult, op1=ALU.add,
            )
            s["ty"] = ty

        def emit_it0B(g):
            base = g * GRP
            s = st[g]
            v0, ty = s.pop("v0"), s["ty"]
            psb = psa_pool.tile([N, GRP, N], F32, tag="psa", name=f"psb{base}_0")
            nc.tensor.matmul(  # psb = -1.5I on all j (512-wide const seed)
                psb[:], lhsT=cm15bt[:],
                rhs=idb[:].unsqueeze(1).broadcast_to([N, GRP, N]),
                start=True, stop=False, skip_group_check=True,
            )
            for j in range(GRP):
                nc.tensor.matmul(  # += V0 u0  => psb = -1.5I + V0u0
                    psb[:, j, :], lhsT=v0[:, j, :], rhs=ty[:, j, 2 * N :],
                    start=False, stop=True, skip_group_check=True,
                )
            # T1 = 1.5I + 1.5u0 - V0u0 = 1.5*u0 - psb
            nc.vector.scalar_tensor_tensor(
                out=ty[:, :, 0:N], in0=ty[:, :, 2 * N :], scalar=1.5,
                in1=psb[:], op0=ALU.mult, op1=ALU.subtract,
            )

        def emit_itA(g, it):
            base = g * GRP
            s = st[g]
            ty = s["ty"]
            psa = psa_pool.tile(
                [N, GRP, N], F32, tag="psa", name=f"psa{base}_{it}"
            )
            for j in range(GRP):
                nc.tensor.matmul(
                    psa[:, j, :], lhsT=ty[:, j, 0:N], rhs=ty[:, j, 0:N],
                    start=True, stop=True,
                )
            if it < NUM_ITER - 2:
                nc.vector.scalar_tensor_tensor(
                    out=ty[:, :, N : 2 * N], in0=ty[:, :, 0:N], scalar=-1.5,
                    in1=psa[:], op0=ALU.mult, op1=ALU.add,
                )
            else:
                # u3 = 1.5 T - T^2 = -u: makes psb3 = -1.5I - T u = -T4 so
                # psf = (-T4)(-Yh4) = +Yh5 and the out scale is +2 sqrt(nrm)
                nc.vector.scalar_tensor_tensor(
                    out=ty[:, :, N : 2 * N], in0=ty[:, :, 0:N], scalar=1.5,
                    in1=psa[:], op0=ALU.mult, op1=ALU.subtract,
                )
            tyn = ty_pool.tile(
                [N, GRP, 3 * N], F16, tag="ty", name=f"ty{base}_{it + 1}"
            )
            s["tyn"] = tyn

        def emit_itB(g, it):
            base = g * GRP
            s = st[g]
            ty, tyn = s["ty"], s["tyn"]
            psb = psb_pool.tile(
                [N, GRP, 2 * N], F32, tag="psb", name=f"psb{base}_{it}"
            )
            seed = c15bt if it < NUM_ITER - 2 else cm15bt
            for h in range(2):  # seed halves: matmul out must stay in-bank
                nc.tensor.matmul(  # psb T'-halves = +-1.5I on j pair
                    psb[:, 2 * h : 2 * h + 2, 0:N], lhsT=seed[:],
                    rhs=idb[:].unsqueeze(1).broadcast_to([N, 2, N]),
                    start=True, stop=False, skip_group_check=True,
                )
            for j in range(GRP):
                nc.tensor.matmul(  # += T.[u|Y] => psb = [T' | Y']
                    psb[:, j, :], lhsT=ty[:, j, 0:N], rhs=ty[:, j, N:],
                    start=False, stop=True, skip_group_check=True,
                )
            # one copy: T' -> tyn[0:N], Y' -> tyn[2N:3N] (2-chunk out AP)
            tyn_tu = tyn[:, :, 0 : 3 * N].rearrange(
                "p b (c n) -> p b c n", c=3
            )[:, :, 0::2, :]
            nc.scalar.copy(tyn_tu, psb[:])
            s["ty"] = tyn
            del s["tyn"]

        def emit_finA(g):
            base = g * GRP
            s = st[g]
            ty = s.pop("ty")
            psf = psa_pool.tile([N, GRP, N], F32, tag="psa", name=f"psf{base}")
            for j in range(GRP):
                nc.tensor.matmul(
                    psf[:, j, :], lhsT=ty[:, j, 0:N], rhs=ty[:, j, 2 * N :],
                    start=True, stop=True,
                )
            s["psf"] = psf

        def emit_finB(g):
            base = g * GRP
            s = st.pop(g)
            psf, s2 = s["psf"], s["s2"]
            outg = out_pool.tile([N, GRP, N], F32, tag="outq", name=f"out{base}")
            # out = +2*sqrt(nrm) * psf  (signs cancel: psf = (-T4)(-Yh4))
            # split 3 j on DVE / 1 j on ACT to balance the two engines
            nc.vector.scalar_tensor_tensor(
                out=outg[:, 0:3, :], in0=psf[:, 0:3, :], scalar=1.0,
                in1=s2[:, 0:3].unsqueeze(2).broadcast_to([N, 3, N]),
                op0=ALU.mult, op1=ALU.mult,
            )
            nc.scalar.activation(
                outg[:, 3, :], psf[:, 3, :], AF.Copy, scale=s2[:, 3:4],
            )
            nc.sync.dma_start(
                o[base : base + GRP].rearrange("b p f -> p b f"), outg[:]
            )

        # --- staggered pipeline ------------------------------------------
        # offsets: dma@0 rowsq@1 norm@2 it0A@3 it0B@4 it1A@5 it1B@6
        #          it2A@7 it2B@8 it3A@9 it3B@10 finA+finB@11
        # finA's psf is consumed by finB's STT at the DVE queue tail, so
        # both fit in one tick (psf is ready ~mid-tick on PE, the STT runs
        # last on DVE) -- one less pipeline stage of fill/drain.
        # Emission order within a tick shapes each engine's queue: finB
        # first (frees psf), then rowsq/norm scalars, the A-stages (psa
        # producers early), B-stages, finA, V0 late (GpS tail), dma last.
        DEPTH = 11
        # Fill compression: the first FASTF groups enter two per tick, so
        # the bottleneck engines saturate early in the ramp instead of
        # idling while group 0 trickles through the 12-stage chain. The
        # execution is self-timed dataflow; overfilled early ticks just
        # backpressure through the tile pools.
        FASTF = int(os.environ.get("ASQRT_FASTF", "0"))
        FASTF = min(FASTF - FASTF % 2, ngrp)

        def start_tick(g):
            return g // 2 if g < FASTF else g - FASTF // 2

        from collections import defaultdict

        tick_groups = defaultdict(list)
        for g in range(ngrp):
            tick_groups[start_tick(g)].append(g)
        last = start_tick(ngrp - 1)

        stages = [
            (emit_rowsq, 1),
            (emit_norm, 2),
            (emit_it0A, 3),
            (lambda g: emit_itA(g, 1), 5),
            (lambda g: emit_itA(g, 2), 7),
            (lambda g: emit_itA(g, 3), 9),
            (emit_it0B, 4),
            (lambda g: emit_itB(g, 1), 6),
            (lambda g: emit_itB(g, 2), 8),
            (lambda g: emit_itB(g, 3), 10),
            (emit_finA, 11),
            (emit_v0, 2),
            (emit_finB, 11),
            (emit_dma, 0),
        ]
        for t in range(last + DEPTH + 1):
            for fn, off in stages:
                for g in tick_groups.get(t - off, ()):
                    fn(g)

    nc.compile()
    return nc


def _get_nc():
    dt_mm = os.environ.get("ASQRT_DTYPE", "f32r")
    if dt_mm not in _CACHE:
        _CACHE[dt_mm] = _build(dt_mm)
    return _CACHE[dt_mm]


def kernel(A: np.ndarray) -> np.ndarray:
    global LAST_EXEC_NS
    from concourse.bass_utils import run_bass_kernel_spmd

    nc = _get_nc()
    A2 = np.ascontiguousarray(A.reshape(-1, N, N), dtype=np.float32)
    consts = const_inputs()
    in_maps = [
        {"a": A2[i * NMAT : (i + 1) * NMAT], **consts}
        for i in range(NCORES)
    ]
    trace = os.environ.get("ASQRT_TRACE", "0") == "1"
    res = run_bass_kernel_spmd(nc, in_maps, list(range(NCORES)), trace=trace)
    LAST_EXEC_NS = res.exec_time_ns
    out = np.concatenate([r["o"] for r in res.results], axis=0)
    return out.reshape(B_S, C_DIM, N, N)


if __name__ == "__main__":
    rng = np.random.default_rng(0)
    A = rng.standard_normal((B_S, C_DIM, N, N)).astype(np.float32)
    A = np.einsum("bcij,bckj->bcik", A, A) / N + 1e-3 * np.eye(N, dtype=np.float32)
    out = kernel(A)
    print("ok", out.shape, LAST_EXEC_NS)
